# revision 6
# baseline (speedup 1.0000x reference)
"""Trainium2 Bass kernel for the AverageTreatmentEffect (TPR-parity) loss.

Math (faithful to the reference):
    p   = sigmoid(out)                       [N] f32
    eq  = (float(y) == p)                    exact f32 equality
    pos = (y == 1), prot = (sensitive == 0)
    tp/fn counts per group -> tpr_p, tpr_n -> mu -> relu(M@mu) -> dot(gap,gap)

Only 4 global sums are needed (all counts are recoverable from them):
    d  = sum(y)
    t2 = sum(y * s)
    t3 = sum((p == 1.0) * y)        # == sum(eq & pos): y==1 => eq <=> p==1.0
    t4 = sum((p == 1.0) * y * s)
with
    tp_p = t3 - t4        fn_p = d - t2 - t3 + t4
    tp_n = t4             fn_n = t2 - t4

Sharding: data-parallel over 8 NeuronCores, 1,048,576 elements/core.
Each core streams its shard of (out, y, s) from HBM (12 MB/core; X is
unused by the math and never touched), computes per-partition partial
sums on-chip (ACT: sigmoid + y-convert with accumulate; DVE: fused
multiply/compare+reduce), and returns tiny per-partition partial-sum
tiles. The host sums the 8 tiles (exact integer arithmetic) and runs
the trivial 4x3 epilogue in float32.
"""

import numpy as np

import concourse.bass as bass
import concourse.mybir as mybir
from concourse.tile import TileContext
from concourse.bass_utils import run_bass_kernel_spmd

AFT = mybir.ActivationFunctionType
ALU = mybir.AluOpType

# --- walrus compatibility pass -------------------------------------------
# This container's walrus build rejects (a) instructions with more than one
# sync-wait condition ("Too many sync wait commands") and (b) the
# EVENT_SEMAPHORE_RANGE_CLEAR raw-ISA instruction Tile emits at context exit
# ("ISA wrong length").  Rewrite the module in place: move excess waits onto
# same-engine InstNoOp instructions inserted immediately before (identical
# engine-stream position => identical semantics), and expand the range-clear
# into one InstEventSemaphore "sem-wr-imm 0" per semaphore.
MAX_WAITS = 1


def walrus_fix(nc, max_waits=MAX_WAITS):
    isa176 = nc.isa.Opcode.NEURON_ISA_TPB_OPCODE_EVENT_SEMAPHORE_RANGE_CLEAR.value
    n_nops = 0
    n_clears = 0
    for fn in nc.m.functions:
        for bb in fn.blocks:
            out = []
            for inst in bb.instructions:
                if getattr(inst, "isa_opcode", None) == isa176:
                    ad = inst.ant_dict
                    for semid in range(ad["range_first"], ad["range_last"] + 1):
                        out.append(mybir.InstEventSemaphore(
                            name=f"{inst.name}-wr{semid}",
                            engine=inst.engine,
                            bass_nofuse=True,
                            sync_info=mybir.SyncInfo(
                                on_wait=[],
                                on_update=[mybir.SyncUpdate(
                                    sync_type="semaphore", id=semid,
                                    update_mode="sem-wr-imm", update_value=0)],
                            ),
                        ))
                        nc.register_instruction(out[-1])
                        n_clears += 1
                    continue
                si = inst.sync_info
                if si is not None and len(si.on_wait) > max_waits:
                    waits = list(si.on_wait)
                    while len(waits) > max_waits:
                        chunk, waits = waits[:max_waits], waits[max_waits:]
                        out.append(mybir.InstNoOp(
                            name=f"{inst.name}-w{n_nops}",
                            engine=inst.engine,
                            bass_nofuse=True,
                            sync_info=mybir.SyncInfo(on_wait=chunk, on_update=[]),
                        ))
                        nc.register_instruction(out[-1])
                        n_nops += 1
                    si.on_wait = waits
                out.append(inst)
            bb.instructions[:] = out
    return n_nops, n_clears
# -------------------------------------------------------------------------

N = 8388608
NCORES = 8
P = 128
FC = 512                       # free-dim per chunk
NCHUNK = (N // NCORES) // (P * FC)   # 16
ROWS = NCHUNK * P
IO_BUFS = 8
WK_BUFS = 3

LAST_RESULTS = None
_NC_CACHE = None


def build_nc():
    nc = bass.Bass(trn_type="TRN2")
    lg = nc.dram_tensor("lg", [ROWS, FC], mybir.dt.float32, kind="ExternalInput")
    yv = nc.dram_tensor("yv", [ROWS, FC], mybir.dt.int32, kind="ExternalInput")
    sv = nc.dram_tensor("sv", [ROWS, FC], mybir.dt.int32, kind="ExternalInput")
    acc_out = nc.dram_tensor("acc", [4, P, NCHUNK], mybir.dt.float32,
                             kind="ExternalOutput")

    with TileContext(nc) as tc:
        with (
            tc.tile_pool(name="io", bufs=IO_BUFS) as io,
            tc.tile_pool(name="wk", bufs=WK_BUFS) as wk,
            tc.tile_pool(name="accp", bufs=1) as accp,
        ):
            # One accumulator tile per sum; each written by exactly one engine.
            acc_d = accp.tile([P, NCHUNK], mybir.dt.float32)   # ACT
            acc_t2 = accp.tile([P, NCHUNK], mybir.dt.float32)  # DVE
            acc_t3 = accp.tile([P, NCHUNK], mybir.dt.float32)  # DVE
            acc_t4 = accp.tile([P, NCHUNK], mybir.dt.float32)  # DVE
            for c in range(NCHUNK):
                rows = slice(c * P, (c + 1) * P)
                lgt = io.tile([P, FC], mybir.dt.float32, tag="lgt")
                yt = io.tile([P, FC], mybir.dt.int32, tag="yt")
                st = io.tile([P, FC], mybir.dt.int32, tag="st")
                nc.sync.dma_start(lgt[:], lg[rows, :])
                nc.sync.dma_start(yt[:], yv[rows, :])
                nc.sync.dma_start(st[:], sv[rows, :])

                p = wk.tile([P, FC], mybir.dt.float32, tag="p")
                yconv = wk.tile([P, FC], mybir.dt.float32, tag="yconv")
                ys = wk.tile([P, FC], mybir.dt.float32, tag="ys")
                dead = wk.tile([P, FC], mybir.dt.float32, tag="dead")
                dead2 = wk.tile([P, FC], mybir.dt.float32, tag="dead2")

                # ACT: p = sigmoid(logits)
                nc.scalar.activation(p[:], lgt[:], AFT.Sigmoid)
                # ACT: f32(y), accumulate -> d
                nc.scalar.activation(yconv[:], yt[:], AFT.Copy,
                                     accum_out=acc_d[:, c:c + 1])
                # DVE: ys = y*s (int in, f32 out), accumulate -> t2
                # (tensor_tensor_reduce lowers to a raw-ISA encoding this
                # walrus build rejects; STT with op0=bypass is equivalent)
                nc.vector.scalar_tensor_tensor(
                    out=ys[:], in0=yt[:], scalar=0.0, in1=st[:],
                    op0=ALU.bypass, op1=ALU.mult,
                    accum_out=acc_t2[:, c:c + 1])
                # DVE: (p == 1.0) * y, accumulate -> t3
                nc.vector.scalar_tensor_tensor(
                    out=dead[:], in0=p[:], scalar=1.0, in1=yt[:],
                    op0=ALU.is_equal, op1=ALU.mult,
                    accum_out=acc_t3[:, c:c + 1])
                # DVE: (p == 1.0) * (y*s), accumulate -> t4
                nc.vector.scalar_tensor_tensor(
                    out=dead2[:], in0=p[:], scalar=1.0, in1=ys[:],
                    op0=ALU.is_equal, op1=ALU.mult,
                    accum_out=acc_t4[:, c:c + 1])

            nc.sync.dma_start(acc_out[0], acc_d[:])
            nc.sync.dma_start(acc_out[1], acc_t2[:])
            nc.sync.dma_start(acc_out[2], acc_t3[:])
            nc.sync.dma_start(acc_out[3], acc_t4[:])
    walrus_fix(nc)
    return nc


def _get_nc():
    global _NC_CACHE
    if _NC_CACHE is None:
        _NC_CACHE = build_nc()
    return _NC_CACHE


def _epilogue(d, t2, t3, t4):
    f = np.float32
    tp_p = f(t3 - t4)
    fn_p = f(d - t2 - t3 + t4)
    tp_n = f(t4)
    fn_n = f(t2 - t4)

    def tpr(tp, fn):
        denom = f(tp + fn)
        if denom == f(0.0):
            return f(0.0)
        return f(tp / max(denom, f(1.0)))

    tpr_p = tpr(tp_p, fn_p)
    tpr_n = tpr(tp_n, fn_n)
    mu = np.array([tpr_n, tpr_p, tpr_p], dtype=np.float32)
    M = np.array([[1.0, 0.0, -1.0],
                  [-1.0, 0.0, 1.0],
                  [1.0, 0.0, -1.0],
                  [-1.0, 0.0, 1.0]], dtype=np.float32)
    gap = np.maximum(M @ mu, f(0.0)).astype(np.float32)
    return np.asarray(f(1.0) * np.dot(gap, gap), dtype=np.float32)


def kernel(X=None, out=None, sensitive=None, y=None):
    global LAST_RESULTS
    nc = _get_nc()

    lg = np.ascontiguousarray(out, dtype=np.float32).reshape(NCORES, ROWS, FC)
    yv = np.ascontiguousarray(y, dtype=np.int32).reshape(NCORES, ROWS, FC)
    sv = np.ascontiguousarray(sensitive, dtype=np.int32).reshape(NCORES, ROWS, FC)

    in_maps = [{"lg": lg[i], "yv": yv[i], "sv": sv[i]} for i in range(NCORES)]
    res = run_bass_kernel_spmd(nc, in_maps, core_ids=list(range(NCORES)))
    LAST_RESULTS = res

    # acc: [4, P, NCHUNK] per core = (d, t2, t3, t4) partials.
    totals = np.zeros(4, dtype=np.float64)
    for r in res.results:
        totals += r["acc"].astype(np.float64).sum(axis=(1, 2))
    d, t2, t3, t4 = totals
    return _epilogue(d, t2, t3, t4)


# revision 9
# speedup vs baseline: 1.0232x; 1.0232x over previous
"""Trainium2 Bass kernel for the AverageTreatmentEffect (TPR-parity) loss.

Math (faithful to the reference):
    p   = sigmoid(out)                       [N] f32
    eq  = (float(y) == p)                    exact f32 equality
    pos = (y == 1), prot = (sensitive == 0)
    tp/fn counts per group -> tpr_p, tpr_n -> mu -> relu(M@mu) -> dot(gap,gap)

Only 4 global sums are needed (all counts are recoverable from them):
    d  = sum(y)
    t2 = sum(y * s)
    t3 = sum((p == 1.0) * y)        # == sum(eq & pos): y==1 => eq <=> p==1.0
    t4 = sum(eqy * s)               # eqy = (p == 1.0)*y, t3's own output
with
    tp_p = t3 - t4        fn_p = d - t2 - t3 + t4
    tp_n = t4             fn_n = t2 - t4

Sharding: data-parallel over 8 NeuronCores, 1,048,576 elements/core.
Each core streams its shard of (out, y, s) from HBM (12 MB/core; X is
unused by the math and never touched). Timeline-profiled layout: 17
chunks (15x512 + 2x256 free-dim; the taper shortens the post-stream
compute tail), ACT does sigmoid + y-convert-with-accumulate (d), DVE
does the t3/t4 fused compare-multiply-accumulate passes, GPSIMD takes
the y*s pass for the final chunks so DVE is drained when the last
bytes land, and all 68 partial columns leave in ONE store DMA (four
separate stores serialized ~625ns each on the HWDGE ring). The host
sums the 8 tiny [128, 68] tiles (exact integer arithmetic) and runs
the trivial 4x3 epilogue in float32.
"""

import numpy as np

import concourse.bass as bass
import concourse.mybir as mybir
from concourse.tile import TileContext
from concourse.bass_utils import run_bass_kernel_spmd

AFT = mybir.ActivationFunctionType
ALU = mybir.AluOpType

# --- walrus compatibility pass -------------------------------------------
# This container's walrus build rejects (a) instructions with more than one
# sync-wait condition ("Too many sync wait commands") and (b) the
# EVENT_SEMAPHORE_RANGE_CLEAR raw-ISA instruction Tile emits at context exit
# ("ISA wrong length").  Rewrite the module in place: move excess waits onto
# same-engine InstNoOp instructions inserted immediately before (identical
# engine-stream position => identical semantics), and expand the range-clear
# into one InstEventSemaphore "sem-wr-imm 0" per semaphore.
MAX_WAITS = 1


def walrus_fix(nc, max_waits=MAX_WAITS):
    isa176 = nc.isa.Opcode.NEURON_ISA_TPB_OPCODE_EVENT_SEMAPHORE_RANGE_CLEAR.value
    n_nops = 0
    n_clears = 0
    for fn in nc.m.functions:
        for bb in fn.blocks:
            out = []
            for inst in bb.instructions:
                if getattr(inst, "isa_opcode", None) == isa176:
                    ad = inst.ant_dict
                    for semid in range(ad["range_first"], ad["range_last"] + 1):
                        out.append(mybir.InstEventSemaphore(
                            name=f"{inst.name}-wr{semid}",
                            engine=inst.engine,
                            bass_nofuse=True,
                            sync_info=mybir.SyncInfo(
                                on_wait=[],
                                on_update=[mybir.SyncUpdate(
                                    sync_type="semaphore", id=semid,
                                    update_mode="sem-wr-imm", update_value=0)],
                            ),
                        ))
                        nc.register_instruction(out[-1])
                        n_clears += 1
                    continue
                si = inst.sync_info
                if si is not None and len(si.on_wait) > max_waits:
                    waits = list(si.on_wait)
                    while len(waits) > max_waits:
                        chunk, waits = waits[:max_waits], waits[max_waits:]
                        out.append(mybir.InstNoOp(
                            name=f"{inst.name}-w{n_nops}",
                            engine=inst.engine,
                            bass_nofuse=True,
                            sync_info=mybir.SyncInfo(on_wait=chunk, on_update=[]),
                        ))
                        nc.register_instruction(out[-1])
                        n_nops += 1
                    si.on_wait = waits
                out.append(inst)
            bb.instructions[:] = out
    return n_nops, n_clears
# -------------------------------------------------------------------------

N = 8388608
NCORES = 8
P = 128
N_PER_CORE = N // NCORES            # 1,048,576
WIDTHS = [512] * 15 + [256, 256]     # per-chunk free-dim; sum == 8192
NCHUNK = len(WIDTHS)                 # 17
# (ys on GPSIMD for late chunks simmed 0.14us faster still, but this walrus
# build rejects scalar_tensor_tensor on Pool: "Instruction engine check
# failed" — so every pass stays on DVE.)
IO_BUFS = 8
WK_BUFS = 3

LAST_RESULTS = None
_NC_CACHE = None


def build_nc():
    nc = bass.Bass(trn_type="TRN2")
    lg = nc.dram_tensor("lg", [N_PER_CORE], mybir.dt.float32, kind="ExternalInput")
    yv = nc.dram_tensor("yv", [N_PER_CORE], mybir.dt.int32, kind="ExternalInput")
    sv = nc.dram_tensor("sv", [N_PER_CORE], mybir.dt.int32, kind="ExternalInput")
    acc_out = nc.dram_tensor("acc", [P, 4 * NCHUNK], mybir.dt.float32,
                             kind="ExternalOutput")
    wmax = max(WIDTHS)

    offs = []
    off = 0
    for w in WIDTHS:
        offs.append(off)
        off += P * w

    def dram_chunk(t, c):
        n = P * WIDTHS[c]
        return t[offs[c]:offs[c] + n].rearrange("(p w) -> p w", p=P)

    with TileContext(nc) as tc:
        with (
            tc.tile_pool(name="io", bufs=IO_BUFS) as io,
            tc.tile_pool(name="wk", bufs=WK_BUFS) as wk,
            tc.tile_pool(name="accp", bufs=1) as accp,
        ):
            # One accumulator tile, q-major columns: [d | t2 | t3 | t4],
            # one column per chunk each -> a single output store DMA.
            acc_sb = accp.tile([P, 4 * NCHUNK], mybir.dt.float32)

            def col(q, c):
                return acc_sb[:, q * NCHUNK + c: q * NCHUNK + c + 1]

            for c, w in enumerate(WIDTHS):
                lgt = io.tile([P, wmax], mybir.dt.float32, tag="lgt")
                yt = io.tile([P, wmax], mybir.dt.int32, tag="yt")
                st = io.tile([P, wmax], mybir.dt.int32, tag="st")
                nc.sync.dma_start(lgt[:, :w], dram_chunk(lg, c))
                nc.sync.dma_start(yt[:, :w], dram_chunk(yv, c))
                nc.sync.dma_start(st[:, :w], dram_chunk(sv, c))

                p = wk.tile([P, wmax], mybir.dt.float32, tag="p")
                yconv = wk.tile([P, wmax], mybir.dt.float32, tag="yconv")
                ys = wk.tile([P, wmax], mybir.dt.float32, tag="ys")
                dead = wk.tile([P, wmax], mybir.dt.float32, tag="dead")
                dead2 = wk.tile([P, wmax], mybir.dt.float32, tag="dead2")

                # ACT: p = sigmoid(logits)
                nc.scalar.activation(p[:, :w], lgt[:, :w], AFT.Sigmoid)
                # ACT: f32(y), accumulate -> d
                nc.scalar.activation(yconv[:, :w], yt[:, :w], AFT.Copy,
                                     accum_out=col(0, c))
                # DVE: ys = y*s (int in, f32 out), accumulate -> t2
                nc.vector.scalar_tensor_tensor(
                    out=ys[:, :w], in0=yt[:, :w], scalar=0.0, in1=st[:, :w],
                    op0=ALU.bypass, op1=ALU.mult,
                    accum_out=col(1, c))
                # DVE: eqy = (p == 1.0) * y, accumulate -> t3
                nc.vector.scalar_tensor_tensor(
                    out=dead[:, :w], in0=p[:, :w], scalar=1.0, in1=yt[:, :w],
                    op0=ALU.is_equal, op1=ALU.mult,
                    accum_out=col(2, c))
                # DVE: t4 = sum(eqy * s) — uses t3's output directly so t4
                # does not serialize behind the ys pass.
                nc.vector.scalar_tensor_tensor(
                    out=dead2[:, :w], in0=dead[:, :w], scalar=0.0,
                    in1=st[:, :w], op0=ALU.bypass, op1=ALU.mult,
                    accum_out=col(3, c))

            nc.sync.dma_start(acc_out[:], acc_sb[:])
    walrus_fix(nc)
    return nc


def _get_nc():
    global _NC_CACHE
    if _NC_CACHE is None:
        _NC_CACHE = build_nc()
    return _NC_CACHE


def _epilogue(d, t2, t3, t4):
    f = np.float32
    tp_p = f(t3 - t4)
    fn_p = f(d - t2 - t3 + t4)
    tp_n = f(t4)
    fn_n = f(t2 - t4)

    def tpr(tp, fn):
        denom = f(tp + fn)
        if denom == f(0.0):
            return f(0.0)
        return f(tp / max(denom, f(1.0)))

    tpr_p = tpr(tp_p, fn_p)
    tpr_n = tpr(tp_n, fn_n)
    mu = np.array([tpr_n, tpr_p, tpr_p], dtype=np.float32)
    M = np.array([[1.0, 0.0, -1.0],
                  [-1.0, 0.0, 1.0],
                  [1.0, 0.0, -1.0],
                  [-1.0, 0.0, 1.0]], dtype=np.float32)
    gap = np.maximum(M @ mu, f(0.0)).astype(np.float32)
    return np.asarray(f(1.0) * np.dot(gap, gap), dtype=np.float32)


def kernel(X=None, out=None, sensitive=None, y=None):
    global LAST_RESULTS
    nc = _get_nc()

    lg = np.ascontiguousarray(out, dtype=np.float32).reshape(NCORES, N_PER_CORE)
    yv = np.ascontiguousarray(y, dtype=np.int32).reshape(NCORES, N_PER_CORE)
    sv = np.ascontiguousarray(sensitive, dtype=np.int32).reshape(NCORES, N_PER_CORE)

    in_maps = [{"lg": lg[i], "yv": yv[i], "sv": sv[i]} for i in range(NCORES)]
    res = run_bass_kernel_spmd(nc, in_maps, core_ids=list(range(NCORES)))
    LAST_RESULTS = res

    # acc: [P, 4*NCHUNK] per core, q-major: [d | t2 | t3 | t4] columns.
    totals = np.zeros(4, dtype=np.float64)
    for r in res.results:
        a = r["acc"].astype(np.float64).reshape(P, 4, NCHUNK)
        totals += a.sum(axis=(0, 2))
    d, t2, t3, t4 = totals
    return _epilogue(d, t2, t3, t4)


# revision 11
# speedup vs baseline: 1.0478x; 1.0241x over previous
"""Trainium2 Bass kernel for the AverageTreatmentEffect (TPR-parity) loss.

Math (faithful to the reference):
    p   = sigmoid(out)                       [N] f32
    eq  = (float(y) == p)                    exact f32 equality
    pos = (y == 1), prot = (sensitive == 0)
    tp/fn counts per group -> tpr_p, tpr_n -> mu -> relu(M@mu) -> dot(gap,gap)

Only 4 global sums are needed (all counts are recoverable from them):
    d  = sum(y)
    t2 = sum(y * s)
    t3 = sum((p == 1.0) * y)        # == sum(eq & pos): y==1 => eq <=> p==1.0
    t4 = sum(eqy * s)               # eqy = (p == 1.0)*y, t3's own output
with
    tp_p = t3 - t4        fn_p = d - t2 - t3 + t4
    tp_n = t4             fn_n = t2 - t4

Sharding: data-parallel over 8 NeuronCores, 1,048,576 elements/core.
Each core streams its shard of (out, y, s) from HBM (12 MB/core; X is
unused by the math and never touched). Timeline-profiled layout: 17
chunks (15x512 + 2x256 free-dim; the taper shortens the post-stream
compute tail), ACT does sigmoid + y-convert-with-accumulate (d), DVE
does the t3/t4 fused compare-multiply-accumulate passes, GPSIMD takes
the y*s pass for the final chunks so DVE is drained when the last
bytes land, and all 68 partial columns leave in ONE store DMA (four
separate stores serialized ~625ns each on the HWDGE ring). The host
sums the 8 tiny [128, 68] tiles (exact integer arithmetic) and runs
the trivial 4x3 epilogue in float32.
"""

import numpy as np

import concourse.bass as bass
import concourse.mybir as mybir
from concourse.tile import TileContext
from concourse.bass_utils import run_bass_kernel_spmd

AFT = mybir.ActivationFunctionType
ALU = mybir.AluOpType

# --- walrus compatibility pass -------------------------------------------
# This container's walrus build rejects (a) instructions with more than one
# sync-wait condition ("Too many sync wait commands") and (b) the
# EVENT_SEMAPHORE_RANGE_CLEAR raw-ISA instruction Tile emits at context exit
# ("ISA wrong length").  Rewrite the module in place: move excess waits onto
# same-engine InstNoOp instructions inserted immediately before (identical
# engine-stream position => identical semantics), and expand the range-clear
# into one InstEventSemaphore "sem-wr-imm 0" per semaphore.
MAX_WAITS = 1


def walrus_fix(nc, max_waits=MAX_WAITS):
    isa176 = nc.isa.Opcode.NEURON_ISA_TPB_OPCODE_EVENT_SEMAPHORE_RANGE_CLEAR.value
    n_nops = 0
    n_clears = 0
    for fn in nc.m.functions:
        for bb in fn.blocks:
            out = []
            for inst in bb.instructions:
                if getattr(inst, "isa_opcode", None) == isa176:
                    ad = inst.ant_dict
                    for semid in range(ad["range_first"], ad["range_last"] + 1):
                        out.append(mybir.InstEventSemaphore(
                            name=f"{inst.name}-wr{semid}",
                            engine=inst.engine,
                            bass_nofuse=True,
                            sync_info=mybir.SyncInfo(
                                on_wait=[],
                                on_update=[mybir.SyncUpdate(
                                    sync_type="semaphore", id=semid,
                                    update_mode="sem-wr-imm", update_value=0)],
                            ),
                        ))
                        nc.register_instruction(out[-1])
                        n_clears += 1
                    continue
                si = inst.sync_info
                if si is not None and len(si.on_wait) > max_waits:
                    waits = list(si.on_wait)
                    while len(waits) > max_waits:
                        chunk, waits = waits[:max_waits], waits[max_waits:]
                        out.append(mybir.InstNoOp(
                            name=f"{inst.name}-w{n_nops}",
                            engine=inst.engine,
                            bass_nofuse=True,
                            sync_info=mybir.SyncInfo(on_wait=chunk, on_update=[]),
                        ))
                        nc.register_instruction(out[-1])
                        n_nops += 1
                    si.on_wait = waits
                out.append(inst)
            bb.instructions[:] = out
    return n_nops, n_clears


def hoist_first_dmas(nc, k=6):
    """Move the first k wait-free SP load DMAs from the tile block into the
    main block, before SP's entry-barrier Drain. The HWDGE ring fills while
    the all-engine entry barrier completes, landing the first HBM byte
    ~0.8us earlier. Safe: the hoisted loads carry no waits, write fresh
    SBUF tiles, and their completion semaphores gate compute exactly as
    before (SP's Drain does not wait on DMA completion)."""
    fn = nc.m.functions[0]
    main_bb, tile_bb = fn.blocks[0], fn.blocks[1]
    hoist = []
    for inst in tile_bb.instructions:
        if len(hoist) >= k:
            break
        if inst.opcode == "DMACopy" and inst.engine == mybir.EngineType.SP:
            if inst.sync_info and inst.sync_info.on_wait:
                break
            hoist.append(inst)
    if not hoist:
        return 0
    names = {i.name for i in hoist}
    tile_bb.instructions[:] = [i for i in tile_bb.instructions
                               if i.name not in names]
    idx = next(j for j, inst in enumerate(main_bb.instructions)
               if inst.opcode == "Drain" and inst.engine == mybir.EngineType.SP)
    main_bb.instructions[idx:idx] = hoist
    return len(hoist)


def strip_second_exit_barrier(nc):
    """TileContext exits with [drain-all] -> barrier -> sem-clears ->
    barrier. The second barrier only orders the clears against kernel end;
    engine halt plus NRT's serialization of executions already guarantees
    that, so drop its Drain/EventSemaphore pairs (~0.25us)."""
    fn = nc.m.functions[0]
    insts = fn.blocks[-1].instructions
    last_clear = None
    for j, inst in enumerate(insts):
        si = inst.sync_info
        if (inst.opcode == "EventSemaphore" and si and
                any(u.update_mode == "sem-wr-imm" for u in si.on_update)):
            last_clear = j
    if last_clear is None:
        return 0
    drop = {i.name for i in insts[last_clear + 1:]
            if i.opcode in ("Drain", "EventSemaphore", "NoOp")}
    insts[:] = [i for i in insts if i.name not in drop]
    return len(drop)
# -------------------------------------------------------------------------

N = 8388608
NCORES = 8
P = 128
N_PER_CORE = N // NCORES            # 1,048,576
WIDTHS = [512] * 15 + [256, 256]     # per-chunk free-dim; sum == 8192
NCHUNK = len(WIDTHS)                 # 17
# (ys on GPSIMD for late chunks simmed 0.14us faster still, but this walrus
# build rejects scalar_tensor_tensor on Pool: "Instruction engine check
# failed" — so every pass stays on DVE.)
IO_BUFS = 8
WK_BUFS = 3

LAST_RESULTS = None
_NC_CACHE = None


def build_nc():
    nc = bass.Bass(trn_type="TRN2")
    lg = nc.dram_tensor("lg", [N_PER_CORE], mybir.dt.float32, kind="ExternalInput")
    yv = nc.dram_tensor("yv", [N_PER_CORE], mybir.dt.int32, kind="ExternalInput")
    sv = nc.dram_tensor("sv", [N_PER_CORE], mybir.dt.int32, kind="ExternalInput")
    acc_out = nc.dram_tensor("acc", [P, 4 * NCHUNK], mybir.dt.float32,
                             kind="ExternalOutput")
    wmax = max(WIDTHS)

    offs = []
    off = 0
    for w in WIDTHS:
        offs.append(off)
        off += P * w

    def dram_chunk(t, c):
        n = P * WIDTHS[c]
        return t[offs[c]:offs[c] + n].rearrange("(p w) -> p w", p=P)

    with TileContext(nc) as tc:
        with (
            tc.tile_pool(name="io", bufs=IO_BUFS) as io,
            tc.tile_pool(name="wk", bufs=WK_BUFS) as wk,
            tc.tile_pool(name="accp", bufs=1) as accp,
        ):
            # One accumulator tile, q-major columns: [d | t2 | t3 | t4],
            # one column per chunk each -> a single output store DMA.
            acc_sb = accp.tile([P, 4 * NCHUNK], mybir.dt.float32)

            def col(q, c):
                return acc_sb[:, q * NCHUNK + c: q * NCHUNK + c + 1]

            for c, w in enumerate(WIDTHS):
                lgt = io.tile([P, wmax], mybir.dt.float32, tag="lgt")
                yt = io.tile([P, wmax], mybir.dt.int32, tag="yt")
                st = io.tile([P, wmax], mybir.dt.int32, tag="st")
                nc.sync.dma_start(lgt[:, :w], dram_chunk(lg, c))
                nc.sync.dma_start(yt[:, :w], dram_chunk(yv, c))
                nc.sync.dma_start(st[:, :w], dram_chunk(sv, c))

                p = wk.tile([P, wmax], mybir.dt.float32, tag="p")
                yconv = wk.tile([P, wmax], mybir.dt.float32, tag="yconv")
                ys = wk.tile([P, wmax], mybir.dt.float32, tag="ys")
                dead = wk.tile([P, wmax], mybir.dt.float32, tag="dead")
                dead2 = wk.tile([P, wmax], mybir.dt.float32, tag="dead2")

                # ACT: p = sigmoid(logits)
                nc.scalar.activation(p[:, :w], lgt[:, :w], AFT.Sigmoid)
                # ACT: f32(y), accumulate -> d
                nc.scalar.activation(yconv[:, :w], yt[:, :w], AFT.Copy,
                                     accum_out=col(0, c))
                # DVE: ys = y*s (int in, f32 out), accumulate -> t2
                nc.vector.scalar_tensor_tensor(
                    out=ys[:, :w], in0=yt[:, :w], scalar=0.0, in1=st[:, :w],
                    op0=ALU.bypass, op1=ALU.mult,
                    accum_out=col(1, c))
                # DVE: eqy = (p == 1.0) * y, accumulate -> t3
                nc.vector.scalar_tensor_tensor(
                    out=dead[:, :w], in0=p[:, :w], scalar=1.0, in1=yt[:, :w],
                    op0=ALU.is_equal, op1=ALU.mult,
                    accum_out=col(2, c))
                # DVE: t4 = sum(eqy * s) — uses t3's output directly so t4
                # does not serialize behind the ys pass.
                nc.vector.scalar_tensor_tensor(
                    out=dead2[:, :w], in0=dead[:, :w], scalar=0.0,
                    in1=st[:, :w], op0=ALU.bypass, op1=ALU.mult,
                    accum_out=col(3, c))

            nc.sync.dma_start(acc_out[:], acc_sb[:])
    walrus_fix(nc)
    hoist_first_dmas(nc)
    strip_second_exit_barrier(nc)
    return nc


def _get_nc():
    global _NC_CACHE
    if _NC_CACHE is None:
        _NC_CACHE = build_nc()
    return _NC_CACHE


def _epilogue(d, t2, t3, t4):
    f = np.float32
    tp_p = f(t3 - t4)
    fn_p = f(d - t2 - t3 + t4)
    tp_n = f(t4)
    fn_n = f(t2 - t4)

    def tpr(tp, fn):
        denom = f(tp + fn)
        if denom == f(0.0):
            return f(0.0)
        return f(tp / max(denom, f(1.0)))

    tpr_p = tpr(tp_p, fn_p)
    tpr_n = tpr(tp_n, fn_n)
    mu = np.array([tpr_n, tpr_p, tpr_p], dtype=np.float32)
    M = np.array([[1.0, 0.0, -1.0],
                  [-1.0, 0.0, 1.0],
                  [1.0, 0.0, -1.0],
                  [-1.0, 0.0, 1.0]], dtype=np.float32)
    gap = np.maximum(M @ mu, f(0.0)).astype(np.float32)
    return np.asarray(f(1.0) * np.dot(gap, gap), dtype=np.float32)


def kernel(X=None, out=None, sensitive=None, y=None):
    global LAST_RESULTS
    nc = _get_nc()

    lg = np.ascontiguousarray(out, dtype=np.float32).reshape(NCORES, N_PER_CORE)
    yv = np.ascontiguousarray(y, dtype=np.int32).reshape(NCORES, N_PER_CORE)
    sv = np.ascontiguousarray(sensitive, dtype=np.int32).reshape(NCORES, N_PER_CORE)

    in_maps = [{"lg": lg[i], "yv": yv[i], "sv": sv[i]} for i in range(NCORES)]
    res = run_bass_kernel_spmd(nc, in_maps, core_ids=list(range(NCORES)))
    LAST_RESULTS = res

    # acc: [P, 4*NCHUNK] per core, q-major: [d | t2 | t3 | t4] columns.
    totals = np.zeros(4, dtype=np.float64)
    for r in res.results:
        a = r["acc"].astype(np.float64).reshape(P, 4, NCHUNK)
        totals += a.sum(axis=(0, 2))
    d, t2, t3, t4 = totals
    return _epilogue(d, t2, t3, t4)


# revision 13
# speedup vs baseline: 1.0568x; 1.0086x over previous
"""Trainium2 Bass kernel for the AverageTreatmentEffect (TPR-parity) loss.

Math (faithful to the reference):
    p   = sigmoid(out)                       [N] f32
    eq  = (float(y) == p)                    exact f32 equality
    pos = (y == 1), prot = (sensitive == 0)
    tp/fn counts per group -> tpr_p, tpr_n -> mu -> relu(M@mu) -> dot(gap,gap)

Only 4 global sums are needed (all counts are recoverable from them):
    d  = sum(y)
    t2 = sum(y * s)
    t3 = sum((p == 1.0) * y)        # == sum(eq & pos): y==1 => eq <=> p==1.0
    t4 = sum(eqy * s)               # eqy = (p == 1.0)*y, t3's own output
with
    tp_p = t3 - t4        fn_p = d - t2 - t3 + t4
    tp_n = t4             fn_n = t2 - t4

Sharding: data-parallel over 8 NeuronCores, 1,048,576 elements/core.
Each core streams its shard of (out, y, s) from HBM (12 MB/core; X is
unused by the math and never touched). Timeline-profiled layout: 12
chunks (5x1024 + 5x512 + 2x256 free-dim: big chunks amortize per-DMA
overhead early, the taper shortens the post-stream compute tail); ACT
does sigmoid + y-convert-with-accumulate (d); DVE does the three fused
compare/multiply+accumulate passes; all partial columns leave in ONE
store DMA (separate stores serialize ~625ns each on the HWDGE ring).
Post-passes hoist the first six load DMAs to the very top of the SP
stream (HWDGE fills before the Tile entry barrier / register preamble)
and drop the redundant post-clear exit barrier. The host sums the 8
tiny [128, 48] tiles (exact integer arithmetic) and runs the trivial
4x3 epilogue in float32.
"""

import numpy as np

import concourse.bass as bass
import concourse.mybir as mybir
from concourse.tile import TileContext
from concourse.bass_utils import run_bass_kernel_spmd

AFT = mybir.ActivationFunctionType
ALU = mybir.AluOpType

# --- walrus compatibility pass -------------------------------------------
# This container's walrus build rejects (a) instructions with more than one
# sync-wait condition ("Too many sync wait commands") and (b) the
# EVENT_SEMAPHORE_RANGE_CLEAR raw-ISA instruction Tile emits at context exit
# ("ISA wrong length").  Rewrite the module in place: move excess waits onto
# same-engine InstNoOp instructions inserted immediately before (identical
# engine-stream position => identical semantics), and expand the range-clear
# into one InstEventSemaphore "sem-wr-imm 0" per semaphore.
MAX_WAITS = 1


def walrus_fix(nc, max_waits=MAX_WAITS):
    isa176 = nc.isa.Opcode.NEURON_ISA_TPB_OPCODE_EVENT_SEMAPHORE_RANGE_CLEAR.value
    n_nops = 0
    n_clears = 0
    for fn in nc.m.functions:
        for bb in fn.blocks:
            out = []
            for inst in bb.instructions:
                if getattr(inst, "isa_opcode", None) == isa176:
                    ad = inst.ant_dict
                    for semid in range(ad["range_first"], ad["range_last"] + 1):
                        out.append(mybir.InstEventSemaphore(
                            name=f"{inst.name}-wr{semid}",
                            engine=inst.engine,
                            bass_nofuse=True,
                            sync_info=mybir.SyncInfo(
                                on_wait=[],
                                on_update=[mybir.SyncUpdate(
                                    sync_type="semaphore", id=semid,
                                    update_mode="sem-wr-imm", update_value=0)],
                            ),
                        ))
                        nc.register_instruction(out[-1])
                        n_clears += 1
                    continue
                si = inst.sync_info
                if si is not None and len(si.on_wait) > max_waits:
                    waits = list(si.on_wait)
                    while len(waits) > max_waits:
                        chunk, waits = waits[:max_waits], waits[max_waits:]
                        out.append(mybir.InstNoOp(
                            name=f"{inst.name}-w{n_nops}",
                            engine=inst.engine,
                            bass_nofuse=True,
                            sync_info=mybir.SyncInfo(on_wait=chunk, on_update=[]),
                        ))
                        nc.register_instruction(out[-1])
                        n_nops += 1
                    si.on_wait = waits
                out.append(inst)
            bb.instructions[:] = out
    return n_nops, n_clears


def hoist_first_dmas(nc, k=6):
    """Move the first k wait-free SP load DMAs from the tile block into the
    main block, before SP's entry-barrier Drain. The HWDGE ring fills while
    the all-engine entry barrier completes, landing the first HBM byte
    ~0.8us earlier. Safe: the hoisted loads carry no waits, write fresh
    SBUF tiles, and their completion semaphores gate compute exactly as
    before (SP's Drain does not wait on DMA completion)."""
    fn = nc.m.functions[0]
    main_bb, tile_bb = fn.blocks[0], fn.blocks[1]
    hoist = []
    for inst in tile_bb.instructions:
        if len(hoist) >= k:
            break
        if inst.opcode == "DMACopy" and inst.engine == mybir.EngineType.SP:
            if inst.sync_info and inst.sync_info.on_wait:
                break
            hoist.append(inst)
    if not hoist:
        return 0
    names = {i.name for i in hoist}
    tile_bb.instructions[:] = [i for i in tile_bb.instructions
                               if i.name not in names]
    # insert at the very top of the main block (after the dummy Call), i.e.
    # before SP's register preamble as well — the loads use physical APs and
    # need none of the preamble registers, so HWDGE ring fill starts at t~0.
    main_bb.instructions[1:1] = hoist
    return len(hoist)


def strip_second_exit_barrier(nc):
    """TileContext exits with [drain-all] -> barrier -> sem-clears ->
    barrier. The second barrier only orders the clears against kernel end;
    engine halt plus NRT's serialization of executions already guarantees
    that, so drop its Drain/EventSemaphore pairs (~0.25us)."""
    fn = nc.m.functions[0]
    insts = fn.blocks[-1].instructions
    last_clear = None
    for j, inst in enumerate(insts):
        si = inst.sync_info
        if (inst.opcode == "EventSemaphore" and si and
                any(u.update_mode == "sem-wr-imm" for u in si.on_update)):
            last_clear = j
    if last_clear is None:
        return 0
    drop = {i.name for i in insts[last_clear + 1:]
            if i.opcode in ("Drain", "EventSemaphore", "NoOp")}
    insts[:] = [i for i in insts if i.name not in drop]
    return len(drop)
# -------------------------------------------------------------------------

N = 8388608
NCORES = 8
P = 128
N_PER_CORE = N // NCORES            # 1,048,576
WIDTHS = [1024] * 5 + [512] * 5 + [256, 256]  # per-chunk free-dim; sum == 8192
NCHUNK = len(WIDTHS)                 # 12
# (ys on GPSIMD for late chunks simmed 0.14us faster still, but this walrus
# build rejects scalar_tensor_tensor on Pool: "Instruction engine check
# failed" — so every pass stays on DVE.)
IO_BUFS = 8
WK_BUFS = 3

LAST_RESULTS = None
_NC_CACHE = None


def build_nc():
    nc = bass.Bass(trn_type="TRN2")
    lg = nc.dram_tensor("lg", [N_PER_CORE], mybir.dt.float32, kind="ExternalInput")
    yv = nc.dram_tensor("yv", [N_PER_CORE], mybir.dt.int32, kind="ExternalInput")
    sv = nc.dram_tensor("sv", [N_PER_CORE], mybir.dt.int32, kind="ExternalInput")
    acc_out = nc.dram_tensor("acc", [P, 4 * NCHUNK], mybir.dt.float32,
                             kind="ExternalOutput")
    wmax = max(WIDTHS)

    offs = []
    off = 0
    for w in WIDTHS:
        offs.append(off)
        off += P * w

    def dram_chunk(t, c):
        n = P * WIDTHS[c]
        return t[offs[c]:offs[c] + n].rearrange("(p w) -> p w", p=P)

    with TileContext(nc) as tc:
        with (
            tc.tile_pool(name="io", bufs=IO_BUFS) as io,
            tc.tile_pool(name="wk", bufs=WK_BUFS) as wk,
            tc.tile_pool(name="accp", bufs=1) as accp,
        ):
            # One accumulator tile, q-major columns: [d | t2 | t3 | t4],
            # one column per chunk each -> a single output store DMA.
            acc_sb = accp.tile([P, 4 * NCHUNK], mybir.dt.float32)

            def col(q, c):
                return acc_sb[:, q * NCHUNK + c: q * NCHUNK + c + 1]

            for c, w in enumerate(WIDTHS):
                lgt = io.tile([P, wmax], mybir.dt.float32, tag="lgt")
                yt = io.tile([P, wmax], mybir.dt.int32, tag="yt")
                st = io.tile([P, wmax], mybir.dt.int32, tag="st")
                nc.sync.dma_start(lgt[:, :w], dram_chunk(lg, c))
                nc.sync.dma_start(yt[:, :w], dram_chunk(yv, c))
                nc.sync.dma_start(st[:, :w], dram_chunk(sv, c))

                p = wk.tile([P, wmax], mybir.dt.float32, tag="p")
                yconv = wk.tile([P, wmax], mybir.dt.float32, tag="yconv")
                ys = wk.tile([P, wmax], mybir.dt.float32, tag="ys")
                dead = wk.tile([P, wmax], mybir.dt.float32, tag="dead")
                dead2 = wk.tile([P, wmax], mybir.dt.float32, tag="dead2")

                # ACT: p = sigmoid(logits)
                nc.scalar.activation(p[:, :w], lgt[:, :w], AFT.Sigmoid)
                # ACT: f32(y), accumulate -> d
                nc.scalar.activation(yconv[:, :w], yt[:, :w], AFT.Copy,
                                     accum_out=col(0, c))
                # DVE: ys = y*s (int in, f32 out), accumulate -> t2
                nc.vector.scalar_tensor_tensor(
                    out=ys[:, :w], in0=yt[:, :w], scalar=0.0, in1=st[:, :w],
                    op0=ALU.bypass, op1=ALU.mult,
                    accum_out=col(1, c))
                # DVE: eqy = (p == 1.0) * y, accumulate -> t3
                nc.vector.scalar_tensor_tensor(
                    out=dead[:, :w], in0=p[:, :w], scalar=1.0, in1=yt[:, :w],
                    op0=ALU.is_equal, op1=ALU.mult,
                    accum_out=col(2, c))
                # DVE: t4 = sum(eqy * s) — uses t3's output directly so t4
                # does not serialize behind the ys pass.
                nc.vector.scalar_tensor_tensor(
                    out=dead2[:, :w], in0=dead[:, :w], scalar=0.0,
                    in1=st[:, :w], op0=ALU.bypass, op1=ALU.mult,
                    accum_out=col(3, c))

            nc.sync.dma_start(acc_out[:], acc_sb[:])
    walrus_fix(nc)
    hoist_first_dmas(nc)
    strip_second_exit_barrier(nc)
    return nc


def _get_nc():
    global _NC_CACHE
    if _NC_CACHE is None:
        _NC_CACHE = build_nc()
    return _NC_CACHE


def _epilogue(d, t2, t3, t4):
    f = np.float32
    tp_p = f(t3 - t4)
    fn_p = f(d - t2 - t3 + t4)
    tp_n = f(t4)
    fn_n = f(t2 - t4)

    def tpr(tp, fn):
        denom = f(tp + fn)
        if denom == f(0.0):
            return f(0.0)
        return f(tp / max(denom, f(1.0)))

    tpr_p = tpr(tp_p, fn_p)
    tpr_n = tpr(tp_n, fn_n)
    mu = np.array([tpr_n, tpr_p, tpr_p], dtype=np.float32)
    M = np.array([[1.0, 0.0, -1.0],
                  [-1.0, 0.0, 1.0],
                  [1.0, 0.0, -1.0],
                  [-1.0, 0.0, 1.0]], dtype=np.float32)
    gap = np.maximum(M @ mu, f(0.0)).astype(np.float32)
    return np.asarray(f(1.0) * np.dot(gap, gap), dtype=np.float32)


def kernel(X=None, out=None, sensitive=None, y=None):
    global LAST_RESULTS
    nc = _get_nc()

    lg = np.ascontiguousarray(out, dtype=np.float32).reshape(NCORES, N_PER_CORE)
    yv = np.ascontiguousarray(y, dtype=np.int32).reshape(NCORES, N_PER_CORE)
    sv = np.ascontiguousarray(sensitive, dtype=np.int32).reshape(NCORES, N_PER_CORE)

    in_maps = [{"lg": lg[i], "yv": yv[i], "sv": sv[i]} for i in range(NCORES)]
    res = run_bass_kernel_spmd(nc, in_maps, core_ids=list(range(NCORES)))
    LAST_RESULTS = res

    # acc: [P, 4*NCHUNK] per core, q-major: [d | t2 | t3 | t4] columns.
    totals = np.zeros(4, dtype=np.float64)
    for r in res.results:
        a = r["acc"].astype(np.float64).reshape(P, 4, NCHUNK)
        totals += a.sum(axis=(0, 2))
    d, t2, t3, t4 = totals
    return _epilogue(d, t2, t3, t4)
